# revision 77
# baseline (speedup 1.0000x reference)
"""Block-circulant linear layer (CirculantLinear) Trainium2 kernel.

y = x @ W^T + bias where W is built from a 256x256 grid of 8x8 circulant
blocks given by per-block eigenvalue vectors `eigens` [256, 256, 8].

Math: per-block circulant multiply diagonalizes under the length-8 rFFT:
  Yf[b, i, bin] = sum_j Xf[b, j, bin] * Ef[i, j, bin]
which is, per frequency bin, a [B,256] x [256,256] (complex) matmul —
~4.5x fewer FLOPs than materializing the dense 2048x2048 W.

The length-8 rFFT/irFFT of x/y is a tiny 8x8 host-side BLAS matmul fused
into the (already required) host transpose/pack, so the device runs ONLY
the dense frequency-domain matmuls:

  per 512-row chunk, per core:
    L  : DMA load of the packed-frequency chunk (16 tiles [128, 512] bf16)
    S3 : 56 dense 128x128x512 bf16 matmuls (PSUM f32 accumulate)
         - bins 1..3 (complex pairs): 3 bins x 4 i-tiles x 4 j-tiles
         - bins 0,4 (real):           2 bins x 2 i-tiles x 2 j-tiles
    C  : PSUM -> SBUF copies (f32 -> bf16), alternating scalar/vector
    St : DMA store of the packed-frequency output chunk

Upload row layout (2048 rows, channel-major, batch in the free dim):
  t = 0..11 : bin-pair tiles, t = (P-1)*4 + jq for P in {1,2,3}:
              row t*128 + jj*2 + c holds Xf[:, jq*64+jj, P].(re if c==0 else im)
  t = 12+h  : bin-0 halves: row holds X0[:, h*128 + jj]  (real)
  t = 14+h  : bin-4 halves: row holds X4[:, h*128 + jj]  (real)
Download layout mirrors it with (i, c') in place of (j, c).
"""

import hashlib
import os
import shutil
from contextlib import ExitStack

import ml_dtypes
import numpy as np

import bass_rust
import concourse.bass as bass
import concourse.mybir as mybir
import concourse.tile as tile
from concourse.vector_clock import ScopedClock

BF16 = ml_dtypes.bfloat16

N_CORES = 8
B_FULL, C = 16384, 2048
BPC = B_FULL // N_CORES  # rows per core
BC = 512  # batch chunk
NBLK = 56  # S3 weight blocks


# ---------------------------------------------------------------------------
# Environment patches (applied once on import)
# ---------------------------------------------------------------------------

def _patched_drain_and_barrier(self, tick_clock, wait_clock):
    # The stock version attaches every outstanding sem wait to one SP Drain;
    # this walrus build rejects >1 sync wait on a CTRL instruction, so spread
    # the waits across a chain of drains.
    nc = self.nc
    drain_inst = nc.sync.drain()
    wait_clock.add_sem_waits(
        drain_inst.ins, ScopedClock({None: tick_clock.global_clock})
    )
    si = drain_inst.ins.sync_info
    waits = list(si.on_wait) if si and si.on_wait else []
    if len(waits) > 1:
        si.on_wait = waits[:1]
        # spread the remaining single-wait drains across engines so the
        # exit chain runs in parallel instead of serially on SP
        engs = [nc.sync, nc.scalar, nc.vector, nc.gpsimd, nc.tensor]
        for i in range(1, len(waits)):
            extra = engs[i % len(engs)].drain()
            extra.ins.sync_info = bass_rust.SyncInfo(
                on_wait=waits[i : i + 1], on_update=[]
            )
    nc.all_engine_barrier()
    assert self.sems is not None
    popped = nc._tile_sem_poison_stack.pop()
    assert popped is self._sem_poison
    nc.clear_and_free_semaphores(list(self.sems.allocated().values()))
    nc.all_engine_barrier()


tile.TileContext._drain_and_barrier = _patched_drain_and_barrier

_MAX_WAITS = 1  # this walrus build rejects >1 sync wait per instruction


def _split_sync_waits(nc, maxw=_MAX_WAITS):
    """Walrus here supports few sync waits per instruction; hoist the excess
    onto same-engine NoOps inserted immediately before the instruction."""
    ctr = 0
    for f in nc.m.functions:
        for bb in f.blocks:
            il = bb.instructions
            out = []
            changed = False
            for inst in il:
                si = inst.sync_info
                waits = list(si.on_wait) if si and si.on_wait else []
                if len(waits) > maxw:
                    si.on_wait = waits[:maxw]
                    for i in range(maxw, len(waits), maxw):
                        ctr += 1
                        nop = mybir.InstNoOp(name=f"waitnop-{ctr}", ins=[], outs=[])
                        nop.engine = inst.engine
                        nop.sync_info = bass_rust.SyncInfo(
                            on_wait=waits[i : i + maxw], on_update=[]
                        )
                        out.append(nop)
                    changed = True
                out.append(inst)
            if changed:
                bb.instructions = out


def _install_neff_cache():
    # Persistent on-disk NEFF cache keyed on BIR content: saves the ~3-10 min
    # walrus compile across processes when the kernel is unchanged.
    import concourse.bass2jax as b2j
    from concourse import bass_utils as bu

    orig = bu.compile_bir_kernel
    cache_dir = os.environ.get(
        "BASS_NEFF_CACHE", os.path.join(os.path.expanduser("~"), ".cache", "bass_neff")
    )

    def cached(bir_json, tmpdir, neff_name="file.neff"):
        try:
            os.makedirs(cache_dir, exist_ok=True)
            h = hashlib.sha256(bir_json).hexdigest()[:32]
            src = os.path.join(cache_dir, h + ".neff")
            if os.path.exists(src):
                dst = os.path.join(tmpdir, neff_name)
                shutil.copy(src, dst)
                return dst
            p = orig(bir_json, tmpdir, neff_name)
            shutil.copy(p, src)
            return p
        except OSError:
            return orig(bir_json, tmpdir, neff_name)

    b2j.compile_bir_kernel = cached
    bu.compile_bir_kernel = cached


_install_neff_cache()


# ---------------------------------------------------------------------------
# Host-side pack/unpack (length-8 rFFT folded into the transpose)
# ---------------------------------------------------------------------------

def _make_F8():
    # packed rfft rows: fc=0: X0; fc=1: X4; fc=2m/2m+1: bin m re/im
    F = np.zeros((8, 8), np.float64)
    k = np.arange(8)
    F[0] = 1.0
    F[1] = (-1.0) ** k
    for m in (1, 2, 3):
        F[2 * m] = np.cos(2 * np.pi * m * k / 8)
        F[2 * m + 1] = -np.sin(2 * np.pi * m * k / 8)
    return F


def _make_F8inv():
    Fi = np.zeros((8, 8), np.float64)  # [t, fc]
    t = np.arange(8)
    Fi[:, 0] = 1 / 8
    Fi[:, 1] = ((-1.0) ** t) / 8
    for m in (1, 2, 3):
        Fi[:, 2 * m] = (2 / 8) * np.cos(2 * np.pi * m * t / 8)
        Fi[:, 2 * m + 1] = -(2 / 8) * np.sin(2 * np.pi * m * t / 8)
    return Fi


def pack_x(x):
    """x [B, 2048] f32 -> packed-frequency channel-major upload [2048, B] bf16."""
    b = x.shape[0]
    F8 = _make_F8().astype(np.float32)
    xp = np.asarray(x, np.float32).reshape(b, 256, 8) @ F8.T  # [B, 256, 8 fc]
    xpT = np.ascontiguousarray(xp.transpose(1, 2, 0))  # [256 j, 8 fc, B]
    xup = np.empty((C, b), dtype=BF16)
    for P in (1, 2, 3):
        for jq in range(4):
            t = (P - 1) * 4 + jq
            blk = xpT[jq * 64 : (jq + 1) * 64, 2 * P : 2 * P + 2, :]  # [64, 2, B]
            xup[t * 128 : (t + 1) * 128] = blk.reshape(128, b)
    for h in (0, 1):
        xup[(12 + h) * 128 : (13 + h) * 128] = xpT[h * 128 : (h + 1) * 128, 0, :]
        xup[(14 + h) * 128 : (15 + h) * 128] = xpT[h * 128 : (h + 1) * 128, 1, :]
    return xup


def unpack_y(yt):
    """Packed-frequency channel-major device output [2048, B] bf16 -> y [B, 2048] f32."""
    b = yt.shape[1]
    ytf = np.asarray(yt, np.float32)
    yp = np.empty((b, 256, 8), np.float32)
    for P in (1, 2, 3):
        for iq in range(4):
            t = (P - 1) * 4 + iq
            blk = ytf[t * 128 : (t + 1) * 128].reshape(64, 2, b)
            yp[:, iq * 64 : (iq + 1) * 64, 2 * P : 2 * P + 2] = blk.transpose(2, 0, 1)
    for h in (0, 1):
        yp[:, h * 128 : (h + 1) * 128, 0] = ytf[(12 + h) * 128 : (13 + h) * 128].T
        yp[:, h * 128 : (h + 1) * 128, 1] = ytf[(14 + h) * 128 : (15 + h) * 128].T
    Fi = _make_F8inv().astype(np.float32)
    y = yp @ Fi.T  # [B, 256, 8 t]
    return y.reshape(b, C)


def make_w(eigens):
    """S3 stationaries, packed [128, 56*128] bf16.

    Block order: bins 1..3: idx ((P-1)*4 + iq)*4 + jq, lhsT[(jj,c),(ii,c')]
    = the 2x2 real form of Ef[i, j, P]; bin 0: idx 48 + iq2*2 + jq2 =
    E0[i-half, j-half].T; bin 4: idx 52 + iq2*2 + jq2 = E4 halves.
    """
    Ef = np.fft.rfft(np.asarray(eigens, np.float64), axis=-1)  # [gy, gx, 5]
    w = np.zeros((128, NBLK * 128), np.float64)
    bi = 0
    for P in (1, 2, 3):
        Er, Ei = Ef[:, :, P].real, Ef[:, :, P].imag
        for iq in range(4):
            for jq in range(4):
                sr = Er[iq * 64 : (iq + 1) * 64, jq * 64 : (jq + 1) * 64].T  # [jj, ii]
                si = Ei[iq * 64 : (iq + 1) * 64, jq * 64 : (jq + 1) * 64].T
                blk = np.empty((128, 128), np.float64)
                blk[0::2, 0::2] = sr
                blk[0::2, 1::2] = si
                blk[1::2, 0::2] = -si
                blk[1::2, 1::2] = sr
                w[:, bi * 128 : (bi + 1) * 128] = blk
                bi += 1
    for E in (Ef[:, :, 0].real, Ef[:, :, 4].real):
        for iq2 in range(2):
            for jq2 in range(2):
                w[:, bi * 128 : (bi + 1) * 128] = E[
                    iq2 * 128 : (iq2 + 1) * 128, jq2 * 128 : (jq2 + 1) * 128
                ].T
                bi += 1
    assert bi == NBLK
    return w.astype(BF16)


# ---------------------------------------------------------------------------
# Device kernel
# ---------------------------------------------------------------------------

# (out_block, w_block_base, xb_block_base, n_jq) per output tile, in issue order
def _s3_plan():
    plan = []
    for P in (1, 2, 3):
        for iq in range(4):
            ob = (P - 1) * 4 + iq
            plan.append((ob, ob * 4, (P - 1) * 4, 4))
    for bi, base in ((0, 48), (1, 52)):
        for iq2 in range(2):
            ob = 12 + bi * 2 + iq2
            plan.append((ob, base + iq2 * 2, 12 + bi * 2, 2))
    return plan


def build_nc(rows=BPC, repeat=1, split_waits=True, cfg=None):
    cfg = dict(cfg or {})
    load_engs = cfg.get("load_engs", ["sync"])  # round-robin
    store_eng = cfg.get("store_eng", "scalar")
    w_eng = cfg.get("w_eng", "sync")
    cp = cfg.get(
        "copy",
        ["vector", "scalar"] * 8,  # per output block
    )
    xb_bufs = cfg.get("xb_bufs", 4)
    ys_bufs = cfg.get("ys_bufs", 8)
    ps_bufs = cfg.get("ps_bufs", 7)
    part_order = cfg.get("part_order", [0, 1, 2, 3])
    prefetch = cfg.get("prefetch", 3)  # chunks ahead to load
    wmm = cfg.get("wmm", 40)  # PE-ramp warmup matmuls (128 cols each)

    f32 = mybir.dt.float32
    bf16 = mybir.dt.bfloat16
    plan = list(cfg.get("chunks", [BC] * (rows // BC)))
    assert sum(plan) == rows and all(b % 128 == 0 and b <= BC for b in plan)
    nchunk = len(plan)
    r0 = [sum(plan[:i]) for i in range(nchunk)]
    assert repeat == 1

    s3 = _s3_plan()

    nc = bass.Bass("TRN2", target_bir_lowering=False, debug=False, num_devices=N_CORES)
    xt_d = nc.declare_dram_parameter("xt", [C, rows], bf16, isOutput=False)
    w_d = nc.declare_dram_parameter("w", [128, NBLK * 128], bf16, isOutput=False)
    y_d = nc.declare_dram_parameter("yt", [C, rows], bf16, isOutput=True)

    xt3 = xt_d.ap().rearrange("(t p) r -> p t r", p=128)  # [128, 16, rows]
    yt3 = y_d.ap().rearrange("(t p) r -> p t r", p=128)

    with tile.TileContext(nc) as tc, ExitStack() as ctx:
        cpool = ctx.enter_context(tc.tile_pool(name="consts", bufs=1))
        w = cpool.tile([128, NBLK * 128], bf16)

        xb_pool = ctx.enter_context(tc.tile_pool(name="xb", bufs=xb_bufs))
        ps_pool = ctx.enter_context(tc.tile_pool(name="s3_ps", bufs=ps_bufs, space="PSUM"))
        y_pool = ctx.enter_context(tc.tile_pool(name="ysb", bufs=ys_bufs))

        xb_t = {}
        loaded = set()  # (ch, part) load DMAs already emitted
        nload = [0]

        def emit_wload(lo, hi, eng=None):
            getattr(nc, eng or w_eng).dma_start(
                w[:, lo * 128 : hi * 128], w_d.ap()[:, lo * 128 : hi * 128]
            )

        def emit_load(ch, part, tlo=None, thi=None, cols=None):
            # one quarter of a chunk: 4 t-blocks (bin-pair P=part+1, or 0&4)
            if (ch, part) in loaded and tlo is None:
                return
            bc = plan[ch]
            if ch not in xb_t:
                xb_t[ch] = xb_pool.tile([128, 16 * BC], bf16, name="xbt")
            xb = xb_t[ch]
            xg = xb[:, : 16 * bc].rearrange("p (t b) -> p t b", t=16)
            if tlo is None:
                tlo, thi = part * 4, part * 4 + 4
                loaded.add((ch, part))
            eng = load_engs[nload[0] % len(load_engs)]
            nload[0] += 1
            c0, c1 = cols if cols is not None else (0, bc)
            getattr(nc, eng).dma_start(
                xg[:, tlo:thi, c0:c1],
                xt3[:, tlo:thi, r0[ch] + c0 : r0[ch] + c1],
            )

        def copy(engname, dst, src):
            eng = getattr(nc, engname)
            if engname == "scalar":
                eng.copy(dst, src)
            else:
                eng.tensor_copy(dst, src)

        ndone = [0]

        def emit_s3_part(ch, part, pre_store=None, order=None):
            # part 0..3: output tiles 4*part..4*part+3 (bin P=part+1, or 0&4),
            # then the store DMA for those 4 blocks. `pre_store` is emitted
            # between the copies and the store so prefetch loads outrank
            # stores in DMA priority.
            bc = plan[ch]
            xb = xb_t[ch]
            ysb = y_pool.tile([128, 4 * BC], bf16)
            order = order or part_order
            final_chunk = ch == nchunk - 1
            final_part = final_chunk and part == order[-1]
            ysb3 = ysb[:, : 4 * bc].rearrange("p (k b) -> p k b", k=4)
            for k in range(4):
                ob, wb, xbase, njq = s3[part * 4 + k]
                ps = ps_pool.tile([128, BC], f32)
                # optional: run the very last tile(s) in column halves so the
                # drain (copy + store) overlaps the remaining matmuls
                first_part = ch == 0 and part == part_order[0]
                ksplit = cfg.get("tail_ksplit", 4)  # split tiles k >= this
                halves = (
                    [(0, bc // 2), (bc // 2, bc)]
                    if (final_part and k >= ksplit)
                    or (first_part and cfg.get("early_cols", False))
                    else [(0, bc)]
                )
                for hi, (c0, c1) in enumerate(halves):
                    for jq in range(njq):
                        base = (wb + jq) * 128
                        nc.tensor.matmul(
                            ps[:, c0:c1],
                            w[:, base : base + 128],
                            xb[:, (xbase + jq) * bc + c0 : (xbase + jq) * bc + c1],
                            start=(jq == 0),
                            stop=(jq == njq - 1),
                        )
                    copy(
                        ["vector", "scalar"][hi] if len(halves) > 1
                        else cp[part * 4 + k],
                        ysb[:, k * bc + c0 : k * bc + c1],
                        ps[:, c0:c1],
                    )
                    # final chunk: small stores right after the copies on the
                    # fast HWDGE queues so nothing big drains at the end
                    # (loads are done; gpsimd's serial SWDGE gen would add
                    # ~1.2us per store to the tail). The very last part splits
                    # 3+1 so the final transfer is a single block.
                    if final_part and cfg.get("final_split31", False):
                        bounds = {2: (0, 3), 3: (3, 4)}
                    else:
                        nb = cfg.get("final_nblk", 2)
                        bounds = {k2: (k2 + 1 - nb, k2 + 1) for k2 in range(4)
                                  if (k2 + 1) % nb == 0}
                    if final_chunk and k in bounds:
                        blo, bhi = bounds[k]
                        fpp = cfg.get("final_store_by_part")
                        if fpp is not None:
                            eng = fpp[order.index(part)]
                        else:
                            fse = cfg.get("final_store_engs", ["sync"])
                            eng = fse[(part + k) % len(fse)]
                        getattr(nc, eng).dma_start(
                            yt3[
                                :,
                                part * 4 + blo : part * 4 + bhi,
                                r0[ch] + c0 : r0[ch] + c1,
                            ],
                            ysb3[:, blo:bhi, c0:c1],
                        )
            if pre_store is not None:
                pre_store()
            if not final_chunk:
                getattr(nc, store_eng).dma_start(
                    yt3[:, part * 4 : (part + 1) * 4, r0[ch] : r0[ch] + bc],
                    ysb3[:, :, :],
                )
            ndone[0] += 1
            if all((ch, p) in loaded for p in range(4)) and part == order[-1]:
                xb_t.pop(ch, None)

        # PE p-state warmup: the cost model ramps the tensor engine
        # 0.65 -> 1.2 -> 2.4 GHz over ~3 us of continuous execution, so burn
        # the DMA lead-in on dep-free dummy matmuls (zeroed SBUF tile) to hit
        # full clock before real data lands.
        if wmm:
            xw = cpool.tile([128, 128], bf16)
            getattr(nc, cfg.get("warm_memset_eng", "gpsimd")).memset(xw[:], 0)
            wps_pool = ctx.enter_context(
                tc.tile_pool(name="warm_ps", bufs=1, space="PSUM")
            )
            wps = wps_pool.tile([128, BC], f32)
            for i in range(wmm):
                nc.tensor.matmul(
                    wps[:, :128], xw[:, :128], xw[:, :128], start=True, stop=True
                )

        # Emission order = Tile scheduler priority hints. Lead-in: x part p
        # of chunk 0 goes BEFORE w part p (except p=0) so chunk-0 data is
        # never behind weight traffic; all remaining loads are emitted
        # up-front (xb_bufs gates how far they actually run ahead) and
        # stores fill the DMA slack behind them.
        # chunk-0 streaming: per part, the x load (sync queue) then that
        # part's w in 4-block pieces (scalar queue) — each output tile's
        # stationary lands just before the PE reaches it
        wsplit = cfg.get("wsplit", [4, 8, 12, 16, 24, 32, 40, 48, 56])
        wpieces = [(lo, hi) for lo, hi in zip([0] + wsplit[:-1], wsplit)]
        for pi, part in enumerate(part_order):
            if pi == 0 and cfg.get("early_cols", False):
                # first load in column halves with the first w piece between:
                # the first matmul starts after half the data, right as the
                # PE clock ramp completes
                bc0 = plan[0]
                emit_load(0, part, cols=(0, bc0 // 2))
                emit_wload(*wpieces.pop(0))
                emit_load(0, part, cols=(bc0 // 2, bc0))
                loaded.add((0, part))
            else:
                emit_load(0, part)
            limit = part * 16 + 16 if part < 3 else 56
            while wpieces and wpieces[0][0] < limit:
                emit_wload(*wpieces.pop(0))
        for ch in range(1, min(prefetch, nchunk)):
            for part in part_order:
                emit_load(ch, part)
        # last chunk may use its own part order (e.g. bins 0&4 first so a
        # long bin part finishes last and its copies hide under the matmuls)
        lpo = cfg.get("last_part_order", part_order)
        for ch in range(nchunk):
            for part in part_order if ch + 1 < nchunk else lpo:
                pre = None
                if ch + prefetch < nchunk:
                    pre = lambda c=ch + prefetch, p=part: emit_load(c, p)
                emit_s3_part(ch, part, pre_store=pre,
                             order=part_order if ch + 1 < nchunk else lpo)

    if split_waits:
        _split_sync_waits(nc)
    return nc


# ---------------------------------------------------------------------------
# Host wrapper
# ---------------------------------------------------------------------------


_NC_CACHE = {}


def _get_nc(rows=BPC):
    if rows not in _NC_CACHE:
        _NC_CACHE[rows] = build_nc(rows)
    return _NC_CACHE[rows]


def kernel(x, eigens, bias):
    from concourse.bass_utils import run_bass_kernel_spmd

    xup = pack_x(x)  # [C, B] packed-frequency channel-major, bf16
    bias = np.asarray(bias, np.float32)
    consts = {"w": make_w(eigens)}

    nc = _get_nc(BPC)
    in_maps = [
        {"xt": np.ascontiguousarray(xup[:, i * BPC : (i + 1) * BPC]), **consts}
        for i in range(N_CORES)
    ]
    res = run_bass_kernel_spmd(nc, in_maps, list(range(N_CORES)))
    yt = np.concatenate([r["yt"] for r in res.results], axis=1)  # [C, B]
    y = unpack_y(yt)
    if np.any(bias):
        y = y + bias
    return np.ascontiguousarray(y)
